# revision 1
# baseline (speedup 1.0000x reference)
"""Causal attention (B=2, H=16, L=2048, D=64, fp32) on 8 trn2 NeuronCores.

Sharding: the 32 (batch, head) pairs are split 4-per-core (pure data/head
parallelism, no cross-core comms). Each core runs the same Bass/Tile program
on its own 4 heads.

Device algorithm (per head):
  - Scores are computed TRANSPOSED: S_T[k, q] = sum_d K[k,d] Q[q,d], via
    matmul(lhsT=kT[:, kb*128:+128], rhs=qT[:, q_chunk]) -> PSUM [128k, 512q].
    Inputs are float32r (FP22-reduced fp32): full PE rate for moving dim>=256.
  - Softmax needs NO reductions in this layout: exp() is applied directly
    (fp32 dynamic range covers |scores| <= ~50 without max subtraction), the
    causal mask is applied by zeroing exp values (one 3D-AP affine_select
    covering all diagonal triangles of a group), and the denominator falls
    out of the PV matmul via a ones-column appended to V:
    out_aug[0:64, q] = numerator, out_aug[64, q] = denominator.
  - exp runs mostly on ScalarE; a tunable number of score groups instead use
    a 4-instruction VectorE exp (Schraudolph exponent construction + exact
    mantissa extraction + cubic correction, ~7e-4 max rel err) to balance
    the two engines.
  - Normalize: the denominator row is broadcast across 64 partitions with a
    K=1 matmul against a ones vector, then fast reciprocal and one multiply.
  - Per-head output is d-major ([64, 2048]); the host transposes it back
    during unsharding (pure layout, no math).
"""

import math
import numpy as np
from contextlib import ExitStack

import concourse.bass as bass
import concourse.bacc as bacc
import concourse.mybir as mybir
import concourse.tile as tile
from concourse.bass_utils import run_bass_kernel_spmd

B, H, L, D = 2, 16, 2048, 64
N_CORES = 8
HPC = (B * H) // N_CORES  # heads per core = 4

F32 = mybir.dt.float32
F32R = mybir.dt.float32r
I32 = mybir.dt.int32
EXP = mybir.ActivationFunctionType.Exp

# ---------------------------------------------------------------------------
# Custom DVE ops for the VectorE exp path.
#
# exp(x) = 2^y, y = x*log2(e).  i = int32(y*2^23 + 127*2^23) gives
# u = bitcast_f32(i) = 2^n * (1+m) with n+m = quantized y (exact in bits).
# True value = 2^(n+m) = u * c(w),  w = 1+m in [1,2),
# c(w) = 2^(w-1)/w, approximated by a cubic (max rel err 6.7e-4).
# w is recovered exactly from u's bits: (i & 0x7FFFFF) | 0x3F800000.
# ---------------------------------------------------------------------------
import concourse.dve_ops as dve_ops
from concourse.dve_spec import (
    AluOp,
    Bin,
    C0,
    C1,
    C2,
    C3,
    Spec,
    Src0,
    _spill_c3_to_src1,
    lower,
    _has_src1,
)
from concourse.dve_uop import DveOpSpec

_MANT_MASK_F = float(np.int32(0x007FFFFF).view(np.float32))  # denormal bits
_ONE_F = 1.0  # bits 0x3F800000

# cubic minimax fit of 2^(w-1)/w on [1,2], rel err <= 6.7e-4
_C3_COEF = (1.77561472, -1.37730759, 0.70747583, -0.1064457)
_SCHRAUD_A = float(np.float32(np.log2(math.e) * 2.0**23))
_SCHRAUD_B = float(np.float32(127.0 * 2.0**23))


def _ref_exp_w(in0, in1, s0, s1, imm2):
    bits = np.asarray(in0, np.float32).view(np.int32)
    w = (bits & np.int32(0x007FFFFF)) | np.int32(0x3F800000)
    return w.view(np.float32)


def _ref_exp_p3(in0, in1, s0, s1, imm2):
    # in1 carries c3 (C3 spilled to Src1 as a [P,1] scalar)
    return s0 + in0 * (s1 + in0 * (imm2 + in0 * in1))


def _make_op(name, body, reference):
    spec = Spec(body=body, reference=reference)
    shas = {}
    for ver in ("v3", "v4"):
        s = DveOpSpec(
            name=name,
            opcode=0,
            uops=lower(spec, ver=ver),
            rd1_en=_has_src1(spec),
        )
        shas[ver] = s.sha(ver)
    op = dve_ops.DveOp(name, spec, subdim=False, uops_sha=shas)
    if name not in dve_ops._SUB_OPCODE_FOR_NAME:
        row = max(dve_ops._SUB_OPCODE_FOR_NAME.values()) + 1
        assert row < 0x20
        dve_ops._SUB_OPCODE_FOR_NAME[name] = row
        dve_ops.OPS.append(op)
        dve_ops.CUSTOM_DVE_SPECS[name] = spec
    return op


EXP_W_ANT = _make_op(
    "EXP_W_ANT",
    Bin(AluOp.BITWISE_OR, Bin(AluOp.BITWISE_AND, Src0, C0), C1),
    _ref_exp_w,
)
EXP_P3_ANT = _make_op(
    "EXP_P3_ANT",
    _spill_c3_to_src1(C0 + Src0 * (C1 + Src0 * (C2 + Src0 * C3))),
    _ref_exp_p3,
)


def _j_order(nj):
    # measured-best chunk processing order (ends on a mid-size chunk)
    return [x for x in (0, 2, 3, 1) if x < nj] or list(range(nj))


def build_nc(hpc=HPC, seq=L, dim=D, qw=512, dve_g=0):
    """Build the single-core Bass/Tile program (shared SPMD across cores)."""
    assert seq % qw == 0 and qw % 128 == 0
    nj = seq // qw          # number of q chunks
    gkb = qw // 128         # k-blocks in the diagonal group
    nkb = seq // 128        # total k blocks
    assert hpc % 2 == 0
    npair = hpc // 2

    nc = bacc.Bacc(trn_type="TRN2")
    # head-PAIR packed q/k: pair p rows 0:64 = head 2p, rows 64:128 = head 2p+1
    qT = nc.dram_tensor("qT", [npair, 2 * dim, seq], F32R, kind="ExternalInput")
    kT = nc.dram_tensor("kT", [npair, 2 * dim, seq], F32R, kind="ExternalInput")
    vA = nc.dram_tensor("vA", [hpc, seq, dim + 1], F32R, kind="ExternalInput")
    onesIn = nc.dram_tensor("ones_in", [128, dim], F32R, kind="ExternalInput")
    oT = nc.dram_tensor("oT", [hpc, dim, seq], F32, kind="ExternalOutput")

    with tile.TileContext(nc) as tc, ExitStack() as ctx:
        qk_pool = ctx.enter_context(tc.tile_pool(name="qk", bufs=1))
        v_pool = ctx.enter_context(tc.tile_pool(name="v", bufs=1))
        exp_pool = ctx.enter_context(tc.tile_pool(name="exp", bufs=8))
        expb_pool = ctx.enter_context(tc.tile_pool(name="expb", bufs=5))
        misc_pool = ctx.enter_context(tc.tile_pool(name="misc", bufs=4))
        dve_pool = ctx.enter_context(tc.tile_pool(name="dve", bufs=2))
        const_pool = ctx.enter_context(tc.tile_pool(name="const", bufs=1))
        ps_a = ctx.enter_context(tc.tile_pool(name="ps_a", bufs=1, space="PSUM"))
        ps_b = ctx.enter_context(tc.tile_pool(name="ps_b", bufs=1, space="PSUM"))
        ps_o = ctx.enter_context(tc.tile_pool(name="ps_o", bufs=2, space="PSUM"))

        # --- load inputs; q/k arrive split per q-chunk, highest chunk first
        # (chunks are processed j-descending, and chunk j's diagonal group
        # needs only column block j) ---
        qts = [None] * npair
        kts = [None] * npair
        vas = [None] * hpc
        for p in range(npair):
            qts[p] = qk_pool.tile([2 * dim, seq], F32R, name=f"qt{p}", tag=f"qt{p}")
            kts[p] = qk_pool.tile([2 * dim, seq], F32R, name=f"kt{p}", tag=f"kt{p}")
        for p in range(npair):
            for c in range(nj):
                cs = slice(c * qw, (c + 1) * qw)
                nc.sync.dma_start(kts[p][:, cs], kT[p][:, cs])
                nc.sync.dma_start(qts[p][:, cs], qT[p][:, cs])
        for h in range(hpc):
            vas[h] = v_pool.tile(
                [128, nkb, dim + 1], F32R, name=f"va{h}", tag=f"va{h}"
            )
            nc.sync.dma_start(
                vas[h][:], vA[h].rearrange("(ko ki) d -> ki ko d", ki=128)
            )

        c3ap = const_pool.tile([128, 1], F32, name="c3ap", tag="c3ap")
        nc.vector.memset(c3ap[:], _C3_COEF[3])
        ones = const_pool.tile([128, dim], F32R, name="ones", tag="ones")
        nc.sync.dma_start(ones[:], onesIn[:])

        def dve_exp(ps_slice, et_slice, w):
            """VectorE exp: ps_slice [128, w] (PSUM f32) -> et_slice (f32r)."""
            t_i = dve_pool.tile([128, 3 * qw], I32, name="dve_i", tag="dve_i")[
                :, :w
            ]
            nc.vector.tensor_scalar(
                t_i, ps_slice, _SCHRAUD_A, _SCHRAUD_B,
                mybir.AluOpType.mult, mybir.AluOpType.add,
            )
            u_f = t_i.bitcast(F32)
            w_f = dve_pool.tile([128, 3 * qw], F32, name="dve_w", tag="dve_w")[
                :, :w
            ]
            nc.vector._custom_dve(
                EXP_W_ANT, out=w_f, in0=u_f, s0=_MANT_MASK_F, s1=_ONE_F
            )
            p_f = dve_pool.tile([128, 3 * qw], F32, name="dve_p", tag="dve_p")[
                :, :w
            ]
            nc.vector._custom_dve(
                EXP_P3_ANT,
                out=p_f,
                in0=w_f,
                in1=c3ap[:],
                s0=_C3_COEF[0],
                s1=_C3_COEF[1],
                imm2=_C3_COEF[2],
            )
            nc.vector.tensor_mul(et_slice, u_f, p_f)

        # --- main loop. Heads are processed in PAIRS: the two heads of a
        # pair live on SBUF partitions 0:64 / 64:128, so their score
        # matmuls (contraction dim 64) target disjoint PE row-groups and
        # run CONCURRENTLY on hardware (tile_position auto-derives from
        # base_partition). Score PSUM tiles are shared by the pair:
        #   A tile [128, 2048] = 2 k-blocks x 2 heads x 512 q
        #   B tile [128, 1024] = 1 k-block  x 2 heads x 512 q
        # A(4 banks) + B(2) + PV out x2 (2) = 8 PSUM banks. ---
        for j in _j_order(nj):
            for p in range(npair):
                nblk = gkb * (j + 1)
                qhs = [qts[p][r : r + dim, :] for r in (0, dim)]
                khs = [kts[p][r : r + dim, :] for r in (0, dim)]
                # plan: list of (tag, [kb...], diag) — nondiag k-blocks in
                # alternating A(2 kbs)/B(1 kb) groups, diag k-blocks last
                # as A(2)+A(2).
                plans = []
                s = 0
                gi = 0
                nd = gkb * j
                while s < nd:
                    cnt = 2 if gi % 2 == 0 else 1
                    cnt = min(cnt, nd - s)
                    plans.append(
                        ("A" if cnt == 2 else "B", list(range(s, s + cnt)), False)
                    )
                    s += cnt
                    gi += 1
                for g in range(gkb // 2):
                    plans.append(
                        ("A", [nd + 2 * g, nd + 2 * g + 1], True)
                    )

                # exp_slices[h][kb] = (rhs_slice, off)
                exp_slices = [[None] * nblk for _ in range(2)]
                for tag, kbs, diag in plans:
                    ncol = 2 * len(kbs)  # 512-wide columns used
                    if tag == "A":
                        ps = ps_a.tile([128, 4 * qw], F32, name="psA", tag="psA")
                        et = exp_pool.tile(
                            [128, 4 * qw], F32R, name="etA", tag="etA"
                        )
                    else:
                        ps = ps_b.tile([128, 2 * qw], F32, name="psB", tag="psB")
                        et = expb_pool.tile(
                            [128, 2 * qw], F32R, name="etB", tag="etB"
                        )
                    # score matmuls: heads interleaved so consecutive MMs hit
                    # disjoint PE row groups (base partition 0 vs 64).
                    # Diagonal blocks with off >= 256 are causally TRIMMED and
                    # written at their bank base (one matmul per PSUM bank,
                    # bank-aligned start - both HW rules respected); their exp
                    # then reads only the valid columns via a strided AP.
                    trim = diag and (kbs[0] - gkb * j) * 128 >= 256
                    for u, kb in enumerate(kbs):
                        off = (kb - gkb * j) * 128 if trim else 0
                        for hh in range(2):
                            col = 2 * u + hh
                            nc.tensor.matmul(
                                ps[:, col * qw : (col + 1) * qw - off],
                                lhsT=khs[hh][:, kb * 128 : (kb + 1) * 128],
                                rhs=qhs[hh][:, j * qw + off : (j + 1) * qw],
                                start=True,
                                stop=True,
                            )
                    if trim:
                        # offs are (256,256,384,384) -> valid widths
                        # (256,256,128,128): one strided exp per width pair
                        pp = 4 * qw  # psum/et partition pitch (A tile)
                        for base_col, wv in ((0, qw - 256), (2, qw - 384)):
                            pin = bass.AP(
                                ps.tensor,
                                ps.offset + base_col * qw,
                                [[pp, 128], [qw, 2], [1, wv]],
                            )
                            pout = bass.AP(
                                et.tensor,
                                et.offset + base_col * qw,
                                [[pp, 128], [qw, 2], [1, wv]],
                            )
                            nc.scalar.activation(pout, pin, EXP)
                    else:
                        w = ncol * qw
                        nc.scalar.activation(et[:, :w], ps[:, :w], EXP)
                    for u, kb in enumerate(kbs):
                        off = (kb - gkb * j) * 128 if diag else 0
                        for hh in range(2):
                            col = 2 * u + hh
                            if trim:
                                exp_slices[hh][kb] = (
                                    et[:, col * qw : (col + 1) * qw - off],
                                    off,
                                )
                            else:
                                exp_slices[hh][kb] = (
                                    et[:, col * qw + off : (col + 1) * qw],
                                    off,
                                )
                    if diag:
                        for u, kb in enumerate(kbs):
                            off = (kb - gkb * j) * 128
                            tb = 0 if trim else off
                            for hh in range(2):
                                col = 2 * u + hh
                                sl = et[:, col * qw + tb : col * qw + tb + 128]
                                nc.gpsimd.affine_select(
                                    out=sl,
                                    in_=sl,
                                    compare_op=mybir.AluOpType.is_ge,
                                    fill=0.0,
                                    base=0,
                                    pattern=[[1, 128]],
                                    channel_multiplier=-1,
                                )

                # PV + normalize per head
                for hh in range(2):
                    h = 2 * p + hh
                    po = ps_o.tile([dim + 1, qw], F32, name="ps_o", tag="ps_o")
                    for i, kb in enumerate(range(nblk)):
                        rhs, off = exp_slices[hh][kb]
                        nc.tensor.matmul(
                            po[:, off:],
                            lhsT=vas[h][:, kb, :],
                            rhs=rhs,
                            start=(i == 0),
                            stop=(i == nblk - 1),
                        )
                    oa = misc_pool.tile([dim + 1, qw], F32R, name="oa", tag="oa")
                    nc.vector.tensor_copy(oa[:], po[:])
                    rec = misc_pool.tile([dim, qw], F32, name="rec", tag="rec")
                    pb = ps_o.tile([dim, qw], F32, name="ps_o", tag="ps_o")
                    nc.tensor.matmul(
                        pb[:],
                        lhsT=ones[dim : dim + 1, :],
                        rhs=oa[dim : dim + 1, :],
                        start=True,
                        stop=True,
                    )
                    nc.vector.reciprocal_approx_fast(rec[:], pb[:])
                    ob = misc_pool.tile([dim, qw], F32, name="ob", tag="ob")
                    nc.vector.tensor_mul(ob[:], oa.bitcast(F32)[:dim, :], rec[:])
                    nc.sync.dma_start(oT[h][:, j * qw : (j + 1) * qw], ob[:])
    nc.compile()
    return nc


_NC_CACHE = {}


def _get_nc(key=(HPC, L, D, 512)):
    if key not in _NC_CACHE:
        _NC_CACHE[key] = build_nc(*key)
    return _NC_CACHE[key]


def make_in_maps(q, k, v):
    """Shard + lay out the full [B,H,L,D] inputs into per-core device maps."""
    qf = np.ascontiguousarray(q, dtype=np.float32).reshape(B * H, L, D)
    kf = np.ascontiguousarray(k, dtype=np.float32).reshape(B * H, L, D)
    vf = np.ascontiguousarray(v, dtype=np.float32).reshape(B * H, L, D)
    in_maps = []
    for c in range(N_CORES):
        sl = slice(HPC * c, HPC * (c + 1))
        # [hpc, L, D] -> [hpc, D, L] -> head-pair packed [hpc//2, 2D, L]
        qTc = np.ascontiguousarray(qf[sl].transpose(0, 2, 1)).reshape(
            HPC // 2, 2 * D, L
        )
        kTc = np.ascontiguousarray(kf[sl].transpose(0, 2, 1)).reshape(
            HPC // 2, 2 * D, L
        )
        vAc = np.concatenate(
            [vf[sl], np.ones((HPC, L, 1), dtype=np.float32)], axis=2
        )
        in_maps.append(
            {
                "qT": qTc,
                "kT": kTc,
                "vA": np.ascontiguousarray(vAc),
                "ones_in": np.ones((128, D), dtype=np.float32),
            }
        )
    return in_maps


def gather_output(results):
    """Per-core oT [hpc, D, L] -> full [B, H, L, D]."""
    oT = np.concatenate([r["oT"] for r in results], axis=0)  # [B*H, D, L]
    return np.ascontiguousarray(
        oT.transpose(0, 2, 1).reshape(B, H, L, D).astype(np.float32)
    )


def run(q, k, v, trace=False, **spmd_kwargs):
    nc = _get_nc()
    res = run_bass_kernel_spmd(
        nc,
        make_in_maps(q, k, v),
        core_ids=list(range(N_CORES)),
        trace=trace,
        **spmd_kwargs,
    )
    return gather_output(res.results), res


def kernel(q, k, v):
    out, _ = run(q, k, v)
    return out



# revision 24
# speedup vs baseline: 1.3151x; 1.3151x over previous
"""Causal attention (B=2, H=16, L=2048, D=64, fp32) on 8 trn2 NeuronCores.

Sharding: the 32 (batch, head) pairs are split 4-per-core (pure data/head
parallelism, no cross-core comms). Each core runs the same Bass/Tile program
on its own 4 heads (2 head-PAIRS packed on SBUF partition halves).

Device algorithm (per head pair):
  - Scores TRANSPOSED: S_T[k, q] = sum_d K[k,d] Q[q,d] via
    matmul(lhsT=kT block, rhs=qT cols) -> PSUM, f32r inputs (1 cyc/col for
    moving dim >= 256). Diagonal blocks causally trimmed to widths
    (512, 512, 256, 256) so PSUM banks pack exactly with no internal gaps.
  - exp applied directly (fp32 range covers |s| <= ~50, no max subtraction),
    output in BF16 to SBUF. Most groups on ScalarE; a tunable subset of
    groups use a 4-instruction VectorE exp (Schraudolph + cubic correction)
    to balance engine load.
  - Causal triangle masks: gpsimd affine_select zeroes exp values.
  - PV is computed TRANSPOSED vs the baseline: out[q, d] with
    lhsT = exp-tile 128x128 block (bf16 weights), rhs = [V | 1] (bf16),
    out [128q, 65] f32 accumulated in PSUM over k blocks. bf16 gets
    1 cyc/row at moving dim 65, so PV costs ~65 cyc per 128x128 block.
    The ones-column makes out[:, 64] the softmax denominator, a
    per-PARTITION scalar.
  - 4 q-blocks' [128, 65] outputs pack into ONE PSUM bank (no crossing).
  - Normalize: DVE copies the [128, 260] bank to SBUF, gpsimd
    normalize_recip divides rows by the per-partition denominator.
  - Output is [q, d]-major: DMA'd straight into oT[h] with no transpose.
"""

import math
import numpy as np
from contextlib import ExitStack

import concourse.bass as bass
import concourse.bacc as bacc
import concourse.mybir as mybir
import concourse.tile as tile
from concourse.bass_utils import run_bass_kernel_spmd

B, H, L, D = 2, 16, 2048, 64
N_CORES = 8
HPC = (B * H) // N_CORES  # heads per core = 4

F32 = mybir.dt.float32
F32R = mybir.dt.float32r
F16 = mybir.dt.float16
BF16 = mybir.dt.bfloat16
I32 = mybir.dt.int32
EXP = mybir.ActivationFunctionType.Exp
ALU = mybir.AluOpType

# ---------------------------------------------------------------------------
# Custom DVE op for the VectorE exp path (cubic correction).
# exp(x) = 2^y, y = x*log2(e).  i = int32(y*2^23 + 127*2^23) gives
# u = bitcast_f32(i) = 2^n * (1+m) with n+m = quantized y (exact in bits).
# True value = 2^(n+m) = u * c(w),  w = 1+m in [1,2),
# c(w) = 2^(w-1)/w, cubic minimax fit (max rel err 6.7e-4).
# ---------------------------------------------------------------------------
import concourse.dve_ops as dve_ops
from concourse.dve_spec import (
    AluOp,
    Bin,
    C0,
    C1,
    C2,
    C3,
    Spec,
    Src0,
    _spill_c3_to_src1,
    lower,
    _has_src1,
)
from concourse.dve_uop import DveOpSpec

_MANT_MASK_I = 0x007FFFFF
_ONE_I = 0x3F800000
_C3_COEF = (1.77561472, -1.37730759, 0.70747583, -0.1064457)
_SCHRAUD_A = float(np.float32(np.log2(math.e) * 2.0**23))
_SCHRAUD_B = float(np.float32(127.0 * 2.0**23))


def _ref_exp_p3(in0, in1, s0, s1, imm2):
    # in1 carries c3 (C3 spilled to Src1 as a [P,1] scalar)
    return s0 + in0 * (s1 + in0 * (imm2 + in0 * in1))


def _make_op(name, body, reference):
    spec = Spec(body=body, reference=reference)
    shas = {}
    for ver in ("v3", "v4"):
        s = DveOpSpec(
            name=name,
            opcode=0,
            uops=lower(spec, ver=ver),
            rd1_en=_has_src1(spec),
        )
        shas[ver] = s.sha(ver)
    op = dve_ops.DveOp(name, spec, subdim=False, uops_sha=shas)
    if name not in dve_ops._SUB_OPCODE_FOR_NAME:
        row = max(dve_ops._SUB_OPCODE_FOR_NAME.values()) + 1
        assert row < 0x20
        dve_ops._SUB_OPCODE_FOR_NAME[name] = row
        dve_ops.OPS.append(op)
        dve_ops.CUSTOM_DVE_SPECS[name] = spec
    return op


EXP_P3_ANT = _make_op(
    "EXP_P3_ANT",
    _spill_c3_to_src1(C0 + Src0 * (C1 + Src0 * (C2 + Src0 * C3))),
    _ref_exp_p3,
)


J_ORDER = (0, 2, 3, 1)
WARMUP_MMS = 10


def _j_order(nj):
    return [x for x in J_ORDER if x < nj] or list(range(nj))


GROUP_COLS = 1536  # score ring tile width (3 PSUM banks, bufs=2 -> 6 banks)
NBANKS = GROUP_COLS // 512


def _pack_plan(j, gkb=4, qw=512):
    """Pack the chunk-j score items into GROUP_COLS-wide PSUM ring groups.

    HW rule: every matmul PSUM write starts at a bank base, one matmul per
    bank -- so each item occupies its own 512-col bank. Diagonal blocks are
    exactly trimmed (fp16 scores cost 1 cyc/col at any width): widths
    512/384/256/128 with the two heads' equal-width items in adjacent banks
    so one strided-AP exp instruction covers the pair (no gap cost).

    Returns groups of (items, runs): item = (kb, hh, width, qoff, col);
    run = ("contig", col, width) or ("pair", col, width) [2 items at
    col and col+512].
    """
    items = []
    nd = gkb * j
    for kb in range(nd):
        for hh in range(2):
            items.append((kb, hh, qw, 0))
    for hh in range(2):
        items.append((nd + 0, hh, qw, 0))
    for hh in range(2):
        items.append((nd + 1, hh, 384, 128))
    for hh in range(2):
        items.append((nd + 2, hh, 256, 256))
    for hh in range(2):
        items.append((nd + 3, hh, 128, 384))

    groups = []
    for s0 in range(0, len(items), NBANKS):
        gitems = [
            (kb, hh, w, qoff, 512 * b)
            for b, (kb, hh, w, qoff) in enumerate(items[s0 : s0 + NBANKS])
        ]
        runs = []
        i = 0
        while i < len(gitems):
            w = gitems[i][2]
            if w == qw:
                n = 1
                while i + n < len(gitems) and gitems[i + n][2] == qw:
                    n += 1
                runs.append(("contig", gitems[i][4], n * qw))
                i += n
            elif i + 1 < len(gitems) and gitems[i + 1][2] == w:
                runs.append(("pair", gitems[i][4], w))
                i += 2
            else:
                runs.append(("contig", gitems[i][4], w))
                i += 1
        groups.append((gitems, runs))
    return groups


ETA_BUFS = 20
NORM_ON_POOL = True
PACK_PO = True


def build_nc(hpc=HPC, seq=L, dim=D, qw=512, dve_sel=()):
    """Build the single-core Bass/Tile program (shared SPMD across cores).

    dve_sel: set of (j, p, g) group coordinates whose exp runs on VectorE
    instead of ScalarE (engine load balancing).
    """
    assert seq % qw == 0 and qw % 128 == 0
    nj = seq // qw
    gkb = qw // 128
    nkb = seq // 128
    npair = hpc // 2
    nqb = qw // 128  # q-blocks of 128 per chunk
    dve_sel = set(dve_sel)

    nc = bacc.Bacc(trn_type="TRN2")
    qT = nc.dram_tensor("qT", [npair, 2 * dim, seq], F16, kind="ExternalInput")
    kT = nc.dram_tensor("kT", [npair, 2 * dim, seq], F16, kind="ExternalInput")
    vA = nc.dram_tensor("vA", [hpc, seq, dim + 1], BF16, kind="ExternalInput")
    oT = nc.dram_tensor("oT", [hpc, seq, dim], F32, kind="ExternalOutput")

    with tile.TileContext(nc) as tc, ExitStack() as ctx:
        qk_pool = ctx.enter_context(tc.tile_pool(name="qk", bufs=1))
        v_pool = ctx.enter_context(tc.tile_pool(name="v", bufs=1))
        eta_pool = ctx.enter_context(tc.tile_pool(name="eta", bufs=ETA_BUFS))
        dve_pool = ctx.enter_context(tc.tile_pool(name="dve", bufs=2))
        nsb_pool = ctx.enter_context(tc.tile_pool(name="nsb", bufs=3))
        ob_pool = ctx.enter_context(tc.tile_pool(name="ob", bufs=4))
        const_pool = ctx.enter_context(tc.tile_pool(name="const", bufs=1))
        ps_s = ctx.enter_context(tc.tile_pool(name="ps_s", bufs=2, space="PSUM"))
        ps_o = ctx.enter_context(tc.tile_pool(name="ps_o", bufs=1, space="PSUM"))

        # --- input loads, emitted in stage order so the first compute's
        # data arrives first: a small k piece + the first q chunk lead. ---
        stages = [(j, p) for j in _j_order(nj) for p in range(npair)]
        qts, kts, vas = [None] * npair, [None] * npair, [None] * hpc
        for p in range(npair):
            qts[p] = qk_pool.tile([2 * dim, seq], F16, name=f"qt{p}", tag=f"qt{p}")
            kts[p] = qk_pool.tile([2 * dim, seq], F16, name=f"kt{p}", tag=f"kt{p}")

        def dma_k(p, c0, c1):
            nc.sync.dma_start(kts[p][:, c0:c1], kT[p][:, c0:c1])

        loaded_k = [0] * npair
        v_emitted = False
        for si, (j, p) in enumerate(stages):
            kneed = 128 * gkb * (j + 1)
            if loaded_k[p] == 0:
                dma_k(p, 0, 256)
                loaded_k[p] = 256
            cs = slice(j * qw, (j + 1) * qw)
            nc.sync.dma_start(qts[p][:, cs], qT[p][:, cs])
            while loaded_k[p] < kneed:
                nxt = min(loaded_k[p] + qw, kneed)
                dma_k(p, loaded_k[p], nxt)
                loaded_k[p] = nxt
            if si == 1 and not v_emitted:
                # v is first consumed by PV of stage 0, late in stage 1
                for h in range(hpc):
                    vas[h] = v_pool.tile(
                        [128, nkb, dim + 1], BF16, name=f"va{h}", tag=f"va{h}"
                    )
                    nc.sync.dma_start(
                        vas[h][:],
                        vA[h].rearrange("(ko ki) d -> ki ko d", ki=128),
                    )
                v_emitted = True

        c3ap = const_pool.tile([128, 1], F32, name="c3ap", tag="c3ap")
        nc.vector.memset(c3ap[:], _C3_COEF[3])

        def dve_exp_i1(ps_slice, w):
            """VectorE exp step 1 (the only PSUM reader): releases the
            score ring slot quickly."""
            t_i = dve_pool.tile([128, GROUP_COLS], I32, name="dve_i", tag="dve_i")[
                :, :w
            ]
            nc.vector.tensor_scalar(
                t_i, ps_slice, _SCHRAUD_A, _SCHRAUD_B, ALU.mult, ALU.add
            )
            return t_i

        def dve_exp_rest(t_i, et_slice, w):
            """VectorE exp steps 2-4 (SBUF only), deferred off the ring."""
            t_w = dve_pool.tile([128, GROUP_COLS], I32, name="dve_w", tag="dve_w")[
                :, :w
            ]
            nc.vector.tensor_scalar(
                t_w, t_i, _MANT_MASK_I, _ONE_I, ALU.bitwise_and, ALU.bitwise_or
            )
            u_f = t_i.bitcast(F32)
            w_f = t_w.bitcast(F32)
            p_f = dve_pool.tile([128, GROUP_COLS], F32, name="dve_p", tag="dve_p")[
                :, :w
            ]
            nc.vector._custom_dve(
                EXP_P3_ANT,
                out=p_f,
                in0=w_f,
                in1=c3ap[:],
                s0=_C3_COEF[0],
                s1=_C3_COEF[1],
                imm2=_C3_COEF[2],
            )
            nc.vector.tensor_mul(et_slice, u_f, p_f)

        # --- main loop: software pipeline over (j, p) stages. Stage n's
        # scores+exp are emitted BEFORE stage n-1's PV+normalize so the PE
        # stream interleaves next-stage scores with current-stage PV and
        # ScalarE never drains at chunk boundaries. ---
        def emit_masks(j, gitems, et):
            # causal triangle masks for diagonal items. block k rows sit at
            # chunk-q = db*128 + r; item covers chunk-q [qoff, qoff+w).
            # keep col c iff qoff + c >= db*128 + r, i.e.
            # c*1 + r*(-1) + base >= 0 with base = qoff - db*128.
            nd = gkb * j
            for kb, hh, w, qoff, col in gitems:
                db = kb - nd
                if db < 0:
                    continue
                base = qoff - db * 128
                span = max(0, min(w, 128 - base))
                if span <= 0:
                    continue
                sl = et[:, col : col + span]
                nc.gpsimd.affine_select(
                    out=sl,
                    in_=sl,
                    compare_op=ALU.is_ge,
                    fill=0.0,
                    base=base,
                    pattern=[[1, span]],
                    channel_multiplier=-1,
                )

        def run_aps(ps, et, runs):
            """(psum_ap, et_ap) pairs for the exp runs of a group."""
            out = []
            for kind, col, w in runs:
                if kind == "contig":
                    out.append((ps[:, col : col + w], et[:, col : col + w], w))
                else:  # pair: two equal-width items at col and col+512
                    pin = bass.AP(
                        ps.tensor,
                        ps.offset + col,
                        [[GROUP_COLS, 128], [512, 2], [1, w]],
                    )
                    pout = bass.AP(
                        et.tensor,
                        et.offset + col,
                        [[GROUP_COLS, 128], [512, 2], [1, w]],
                    )
                    out.append((pin, pout, 2 * w))
            return out

        def emit_scores(j, p):
            qhs = [qts[p][r : r + dim, :] for r in (0, dim)]
            khs = [kts[p][r : r + dim, :] for r in (0, dim)]
            et_map = {}
            deferred = []
            for g, (gitems, runs) in enumerate(_pack_plan(j, gkb, qw)):
                ps = ps_s.tile([128, GROUP_COLS], F32, name="psS", tag="psS")
                et = eta_pool.tile([128, GROUP_COLS], BF16, name="et", tag="et")
                for kb, hh, w, qoff, col in gitems:
                    nc.tensor.matmul(
                        ps[:, col : col + w],
                        lhsT=khs[hh][:, kb * 128 : (kb + 1) * 128],
                        rhs=qhs[hh][:, j * qw + qoff : j * qw + qoff + w],
                        start=True,
                        stop=True,
                    )
                    et_map[(kb, hh)] = (et, col, qoff)
                if (j, p, g) in dve_sel:
                    span = gitems[-1][4] + gitems[-1][2]
                    t_i = dve_exp_i1(ps[:, :span], span)
                    deferred.append((t_i, et, span, gitems))
                else:
                    for pin, pout, _w in run_aps(ps, et, runs):
                        nc.scalar.activation(pout, pin, EXP)
                    emit_masks(j, gitems, et)
            return et_map, deferred

        def emit_dve_chains(j, deferred):
            for t_i, et, used, gitems in deferred:
                dve_exp_rest(t_i, et[:, :used], used)
                emit_masks(j, gitems, et)

        def emit_pv(j, p, et_map):
            nd = gkb * j
            nsbs = [
                nsb_pool.tile(
                    [128, nqb * (dim + 1)], F32, name="nsb", tag=f"nsb{hh}"
                )
                for hh in range(2)
            ]
            if PACK_PO:
                pos = [
                    ps_o.tile(
                        [128, nqb * (dim + 1)], F32,
                        name=f"po{hh}", tag=f"po{hh}",
                    )
                    for hh in range(2)
                ]
            # qb-outer / hh-inner: the two po banks alternate so each
            # bank's copy hides under the other head's accumulation chain
            for qb in range(nqb):
                nblk = nd + qb + 1
                for hh in range(2):
                    h = 2 * p + hh
                    if PACK_PO:
                        po_qb = pos[hh][:, qb * (dim + 1) : (qb + 1) * (dim + 1)]
                    else:
                        po_t = ps_o.tile(
                            [128, dim + 1], F32, name=f"po{hh}", tag=f"po{hh}"
                        )
                        po_qb = po_t[:]
                    for i, kb in enumerate(range(nblk)):
                        et, col, qoff = et_map[(kb, hh)]
                        lcol = col + qb * 128 - qoff
                        nc.tensor.matmul(
                            po_qb,
                            lhsT=et[:, lcol : lcol + 128],
                            rhs=vas[h][:, kb, :],
                            start=(i == 0),
                            stop=(i == nblk - 1),
                        )
                    if not PACK_PO:
                        o0 = qb * (dim + 1)
                        nc.vector.tensor_copy(
                            nsbs[hh][:, o0 : o0 + dim + 1], po_t[:]
                        )
            if PACK_PO:
                for hh in range(2):
                    nc.vector.tensor_copy(nsbs[hh][:], pos[hh][:])
            for hh in range(2):
                h = 2 * p + hh
                nsb = nsbs[hh]
                ob = ob_pool.tile([128, nqb, dim], F32, name="ob", tag="ob")
                if not NORM_ON_POOL:
                    rec = nsb_pool.tile([128, nqb], F32, name="rec", tag="rec")
                for qb in range(nqb):
                    o0 = qb * (dim + 1)
                    if NORM_ON_POOL:
                        nc.gpsimd.normalize_recip(
                            ob[:, qb, :],
                            nsb[:, o0 : o0 + dim],
                            nsb[:, o0 + dim : o0 + dim + 1],
                        )
                    else:
                        nc.vector.reciprocal_approx_fast(
                            rec[:, qb : qb + 1],
                            nsb[:, o0 + dim : o0 + dim + 1],
                        )
                        nc.vector.tensor_scalar_mul(
                            ob[:, qb, :],
                            nsb[:, o0 : o0 + dim],
                            rec[:, qb : qb + 1],
                        )
                nc.sync.dma_start(
                    oT[h][j * qw : (j + 1) * qw].rearrange(
                        "(qb np) d -> np qb d", np=128
                    ),
                    ob[:],
                )

        # PE warm-up: dummy matmuls during the initial DMA wait keep the
        # PE pstate ramp going so real matmuls start at full clock.
        wu = const_pool.tile([128, 512], F32, name="wu", tag="wu")
        nc.vector.memset(wu[:], 0.0)
        wu_r = wu.bitcast(F32R)
        wu_ps = ps_s.tile([128, GROUP_COLS], F32, name="psS", tag="psS")
        for _ in range(WARMUP_MMS):
            nc.tensor.matmul(
                wu_ps[0:1, :512],
                lhsT=wu_r[:, 0:1],
                rhs=wu_r[:, :512],
                start=True,
                stop=True,
            )

        prev = None
        for j, p in stages:
            em, deferred = emit_scores(j, p)
            if prev is not None:
                emit_pv(*prev)
            emit_dve_chains(j, deferred)
            prev = (j, p, em)
        emit_pv(*prev)
    nc.compile()
    return nc


_NC_CACHE = {}
DVE_SEL = tuple((j, p, 1) for j in (1, 2, 3) for p in (0, 1))


def _get_nc(key=None):
    if key is None:
        key = (HPC, L, D, 512, tuple(sorted(DVE_SEL)))
    if key not in _NC_CACHE:
        _NC_CACHE[key] = build_nc(*key[:4], dve_sel=key[4])
    return _NC_CACHE[key]


def make_in_maps(q, k, v):
    """Shard + lay out the full [B,H,L,D] inputs into per-core device maps."""
    import ml_dtypes

    qf = np.ascontiguousarray(q, dtype=np.float32).reshape(B * H, L, D).astype(np.float16)
    kf = np.ascontiguousarray(k, dtype=np.float32).reshape(B * H, L, D).astype(np.float16)
    vf = np.ascontiguousarray(v, dtype=np.float32).reshape(B * H, L, D)
    in_maps = []
    for c in range(N_CORES):
        sl = slice(HPC * c, HPC * (c + 1))
        qTc = np.ascontiguousarray(qf[sl].transpose(0, 2, 1)).reshape(
            HPC // 2, 2 * D, L
        )
        kTc = np.ascontiguousarray(kf[sl].transpose(0, 2, 1)).reshape(
            HPC // 2, 2 * D, L
        )
        vAc = np.concatenate(
            [vf[sl], np.ones((HPC, L, 1), dtype=np.float32)], axis=2
        ).astype(ml_dtypes.bfloat16)
        in_maps.append({"qT": qTc, "kT": kTc, "vA": np.ascontiguousarray(vAc)})
    return in_maps


def gather_output(results):
    """Per-core oT [hpc, L, D] -> full [B, H, L, D] (no transpose needed)."""
    oT = np.concatenate([r["oT"] for r in results], axis=0)  # [B*H, L, D]
    return np.ascontiguousarray(oT.reshape(B, H, L, D).astype(np.float32))


def run(q, k, v, trace=False, **spmd_kwargs):
    nc = _get_nc()
    res = run_bass_kernel_spmd(
        nc,
        make_in_maps(q, k, v),
        core_ids=list(range(N_CORES)),
        trace=trace,
        **spmd_kwargs,
    )
    return gather_output(res.results), res


def kernel(q, k, v):
    out, _ = run(q, k, v)
    return out
